# revision 1
# baseline (speedup 1.0000x reference)
"""Trainium2 Bass kernel for MoE-with-LoRA-experts (nn_MoE_64098091925598).

Reference computation (N=8192 tokens, D=1024, E=8 experts, R=16, top-2):
    logits  = x @ W_gate.T                      [N, E]
    combine = scatter(softmax(top2(logits)))    [N, E] (2 nonzeros/row)
    moe     = sum_e combine[:,e] * (x @ A_e @ B_e)
    out     = moe + x @ W_base.T + b_base

Strategy: data-parallel over tokens across 8 NeuronCores (1024 tokens
per core); every core computes all 8 LoRA experts densely (the expert
FLOPs are tiny vs the base linear, so expert-parallel all-to-all would
be pure overhead) and masks by the combine weights. All matmuls run in
float32r (TF32-like fast fp32 path, full PE rate at N>=256).

Key algebraic trick: with H[n,(e,r)] = combine[n,e] * (x @ A_e)[n,r]
stacked over experts, the weighted expert sum collapses to a single
dense K=128 matmul  moe = H @ B_flat,  which accumulates into the same
PSUM tile as the base-linear matmuls.

Layouts (host-prepped): x is transposed to xT [D, N] so the contraction
dim d sits on SBUF partitions; outputs are produced token-major [n, d]
directly, so the gather is a plain concat.

Perf structure:
- Loads stream in priority order with x^T's first 512-token half split
  across BOTH HWDGE rings (sync+scalar) so gating starts ~12us in; the
  W_base halves follow on whichever ring frees up, ordered dt-major to
  feed the dt-outer main loop.
- The gating/LoRA front-end is pipelined per 512-token wave (separate
  tiles per wave so Tile's dependency tracking doesn't serialize).
- A short dummy-matmul burst warms the PE clock gate (HAM) while the
  first loads are in flight.
"""

import numpy as np

import concourse.mybir as mybir
import concourse.tile as tile
from concourse import bacc
from concourse.bass_utils import run_bass_kernel_spmd
from concourse.masks import make_identity

N_TOK, D, E, R, TOPK = 8192, 1024, 8, 16, 2
CORES = 8
NS = N_TOK // CORES  # tokens per core
ER = E * R  # 128, stacked expert-rank dim
DC = D // 128  # 8 contraction chunks
NJ = NS // 128  # 8 token chunks per core
NT = NS // 512  # 2 wide token tiles (waves) per core
JT = NJ // NT  # 4 token chunks per wave
DT = D // 512  # 2 dout tiles

f32 = mybir.dt.float32
f32r = mybir.dt.float32r

N_WARM = 36  # dummy matmuls to warm the PE clock gate during loads

_CACHE: dict = {}


def _kernel_body(nc, tc, dram):
    xT, wbT, a_fl, b_fl, wgT, exp_m, b_vec, out = dram

    from contextlib import ExitStack

    ctx = ExitStack()
    pw = ctx.enter_context(tc.tile_pool(name="weights", bufs=1))
    pg = ctx.enter_context(tc.tile_pool(name="gating", bufs=1))
    pmt = ctx.enter_context(tc.tile_pool(name="mmtmp", bufs=2))
    pout = ctx.enter_context(tc.tile_pool(name="outsb", bufs=4))
    ps_tp = ctx.enter_context(tc.tile_pool(name="ps_tp", bufs=2, space="PSUM"))
    ps_mm = ctx.enter_context(tc.tile_pool(name="ps_mm", bufs=2, space="PSUM"))
    ps_out = ctx.enter_context(tc.tile_pool(name="ps_out", bufs=4, space="PSUM"))

    # ---- PE prewarm: garbage matmuls, no data deps, never read -----
    warm_sb = pw.tile([128, 128], f32r, tag="warm")
    warm_ps = ps_tp.tile([128, 64], f32, tag="tp")
    nc.vector.memset(warm_sb.bitcast(f32), 0.0)
    for _ in range(N_WARM):
        nc.tensor.matmul(
            warm_ps, warm_sb, warm_sb[:, 0:64], start=True, stop=True
        )

    # ---- Load phase: two HWDGE rings, strict priority order ---------
    # ring A = nc.sync, ring B = nc.scalar. x^T wave-0 is split across
    # both rings; then wave-1 + LoRA weights on A while W_base streams
    # on B (dt-major for the dt-outer main loop).
    xT3 = xT.rearrange("(c p) n -> p c n", p=128)
    wbT3 = wbT.rearrange("(c p) d -> p c d", p=128)

    def ring(c):
        return nc.sync if c % 2 == 0 else nc.scalar

    wg_sb = pw.tile([128, DC, E], f32r, tag="wg")
    nc.scalar.dma_start(out=wg_sb, in_=wgT.rearrange("(c p) e -> p c e", p=128))
    exp_sb = pw.tile([E, ER], f32r, tag="expand")
    nc.scalar.dma_start(out=exp_sb, in_=exp_m)

    xt = [[None] * NT for _ in range(DC)]
    for c in range(DC):
        t = pw.tile([128, 512], f32r, tag=f"xt{c}_0")
        ring(c).dma_start(out=t, in_=xT3[:, c, 0:512])
        xt[c][0] = t

    a_sb = pw.tile([128, DC, ER], f32r, tag="a")
    nc.sync.dma_start(out=a_sb, in_=a_fl.rearrange("(c p) r -> p c r", p=128))

    bias_sb = pw.tile([128, D], f32, tag="bias")
    nc.gpsimd.dma_start(out=bias_sb, in_=b_vec.to_broadcast([128, D]))

    b_sb = pw.tile([ER, D], f32r, tag="bflat")
    nc.scalar.dma_start(out=b_sb, in_=b_fl)

    for c in range(DC):
        t = pw.tile([128, 512], f32r, tag=f"xt{c}_1")
        ring(c).dma_start(out=t, in_=xT3[:, c, 512:1024])
        xt[c][1] = t

    wb = [[None] * DT for _ in range(DC)]
    for dt in range(DT):
        for c in range(DC):
            t = pw.tile([128, 512], f32r, tag=f"wb{c}_{dt}")
            ring(c).dma_start(out=t, in_=wbT3[:, c, dt * 512 : (dt + 1) * 512])
            wb[c][dt] = t

    ident = pw.tile([128, 128], f32, tag="ident")
    make_identity(nc, ident)

    # ---- Front-end, pipelined per 512-token wave --------------------
    HT_sb = pg.tile([ER, NS], f32r, tag="HT")
    for t in range(NT):
        sl = slice(t * 512, (t + 1) * 512)

        # logits^T [E, 512] for this wave
        lgT_ps = ps_mm.tile([E, 512], f32, tag="mm")
        for c in range(DC):
            nc.tensor.matmul(
                lgT_ps, wg_sb[:, c, :], xt[c][t], start=(c == 0), stop=(c == DC - 1)
            )
        lgT_sb = pg.tile([E, 512], f32, tag=f"lgT{t}")
        nc.vector.tensor_copy(lgT_sb, lgT_ps)

        # token-major logits chunks + sorted top-8 per token
        lg3 = pg.tile([128, JT, E], f32, tag=f"lg3_{t}")
        mx = pg.tile([128, JT, E], f32, tag=f"mx{t}")
        for r in range(JT):
            tr_ps = ps_tp.tile([128, E], f32, tag="tp")
            nc.tensor.transpose(
                tr_ps, lgT_sb[:, r * 128 : (r + 1) * 128], ident[0:E, 0:E]
            )
            nc.vector.tensor_copy(lg3[:, r, :], tr_ps)
            nc.vector.max(out=mx[:, r, :], in_=lg3[:, r, :])

        # combine = 1{l==v1}*sigmoid(v1-v2) + 1{l==v2}*sigmoid(v2-v1)
        v1 = mx[:, :, 0:1]
        v2 = mx[:, :, 1:2]
        d21 = pg.tile([128, JT, 1], f32, tag=f"d21_{t}")
        nc.vector.tensor_sub(d21, v2, v1)
        w1 = pg.tile([128, JT, 1], f32, tag=f"w1_{t}")
        w2 = pg.tile([128, JT, 1], f32, tag=f"w2_{t}")
        nc.scalar.activation(w2, d21, mybir.ActivationFunctionType.Sigmoid)
        nc.scalar.activation(w1, d21, mybir.ActivationFunctionType.Sigmoid, scale=-1.0)

        eq1 = pg.tile([128, JT, E], f32, tag=f"eq1_{t}")
        eq2 = pg.tile([128, JT, E], f32, tag=f"eq2_{t}")
        cb = pg.tile([128, JT, E], f32, tag=f"cb{t}")
        bs = [128, JT, E]
        nc.vector.tensor_tensor(eq1, lg3, v1.to_broadcast(bs), mybir.AluOpType.is_equal)
        nc.vector.tensor_tensor(eq2, lg3, v2.to_broadcast(bs), mybir.AluOpType.is_equal)
        nc.vector.tensor_tensor(eq1, eq1, w1.to_broadcast(bs), mybir.AluOpType.mult)
        nc.vector.tensor_tensor(eq2, eq2, w2.to_broadcast(bs), mybir.AluOpType.mult)
        nc.vector.tensor_add(cb, eq1, eq2)

        # combine^T [E, 512] via PE transpose per chunk
        cT_sb = pg.tile([E, 512], f32r, tag=f"cT{t}")
        for r in range(JT):
            cT_ps = ps_tp.tile([E, 128], f32, tag="tp")
            nc.tensor.transpose(cT_ps, cb[:, r, :], ident)
            nc.vector.tensor_copy(cT_sb[:, r * 128 : (r + 1) * 128], cT_ps)

        # H^T = (A_flat^T @ x^T) * expand(combine^T)
        h_ps = ps_mm.tile([ER, 512], f32, tag="mm")
        for c in range(DC):
            nc.tensor.matmul(
                h_ps, a_sb[:, c, :], xt[c][t], start=(c == 0), stop=(c == DC - 1)
            )
        h_sb = pmt.tile([ER, 512], f32, tag="hsb")
        nc.vector.tensor_copy(h_sb, h_ps)
        ce_ps = ps_mm.tile([ER, 512], f32, tag="mm")
        nc.tensor.matmul(ce_ps, exp_sb, cT_sb, start=True, stop=True)
        nc.vector.tensor_tensor(HT_sb[:, sl], ce_ps, h_sb, mybir.AluOpType.mult)

    # ---- Main accumulation: out[n,d] = x@Wb^T + H@B_flat + b --------
    for dt in range(DT):
        dsl = slice(dt * 512, (dt + 1) * 512)
        for j in range(NJ):
            jsl = slice(j * 128, (j + 1) * 128)
            jh, jr = divmod(j, JT)
            out_ps = ps_out.tile([128, 512], f32, tag="out")
            for c in range(DC):
                nc.tensor.matmul(
                    out_ps,
                    xt[c][jh][:, jr * 128 : (jr + 1) * 128],
                    wb[c][dt],
                    start=(c == 0),
                    stop=False,
                )
            nc.tensor.matmul(out_ps, HT_sb[:, jsl], b_sb[:, dsl], start=False, stop=True)
            out_sb = pout.tile([128, 512], f32, tag="osb")
            nc.vector.tensor_add(out_sb, out_ps, bias_sb[:, dsl])
            eng = nc.sync if (j + dt) % 2 == 0 else nc.scalar
            eng.dma_start(out=out[jsl, dsl], in_=out_sb)

    ctx.close()


def build_nc():
    nc = bacc.Bacc(
        "TRN2",
        target_bir_lowering=False,
        debug=False,
        enable_asserts=False,
        num_devices=CORES,
    )
    xT = nc.dram_tensor("xT", [D, NS], f32, kind="ExternalInput").ap()
    wbT = nc.dram_tensor("wbT", [D, D], f32, kind="ExternalInput").ap()
    a_fl = nc.dram_tensor("a_fl", [D, ER], f32, kind="ExternalInput").ap()
    b_fl = nc.dram_tensor("b_fl", [ER, D], f32, kind="ExternalInput").ap()
    wgT = nc.dram_tensor("wgT", [D, E], f32, kind="ExternalInput").ap()
    exp_m = nc.dram_tensor("exp_m", [E, ER], f32, kind="ExternalInput").ap()
    b_vec = nc.dram_tensor("b_vec", [1, D], f32, kind="ExternalInput").ap()
    out = nc.dram_tensor("out", [NS, D], f32, kind="ExternalOutput").ap()

    dram = (
        xT.bitcast(f32r),
        wbT.bitcast(f32r),
        a_fl.bitcast(f32r),
        b_fl.bitcast(f32r),
        wgT.bitcast(f32r),
        exp_m.bitcast(f32r),
        b_vec,
        out,
    )
    with tile.TileContext(nc) as tc:
        _kernel_body(nc, tc, dram)
    nc.compile()
    return nc


def host_prep(x, W_gate, A, B, W_base, b_base):
    """Shard + lay out the full inputs into 8 per-core input maps."""
    xT = np.ascontiguousarray(x.T)  # [D, N]
    wbT = np.ascontiguousarray(W_base.T)  # [din, dout]
    a_fl = np.ascontiguousarray(A.transpose(1, 0, 2).reshape(D, ER))
    b_fl = np.ascontiguousarray(B.reshape(ER, D))
    wgT = np.ascontiguousarray(W_gate.T)  # [D, E]
    exp_m = np.zeros((E, ER), dtype=np.float32)
    for e in range(E):
        exp_m[e, e * R : (e + 1) * R] = 1.0
    b_vec = np.ascontiguousarray(b_base.reshape(1, D))

    in_maps = []
    for c in range(CORES):
        in_maps.append(
            {
                "xT": np.ascontiguousarray(xT[:, c * NS : (c + 1) * NS]),
                "wbT": wbT,
                "a_fl": a_fl,
                "b_fl": b_fl,
                "wgT": wgT,
                "exp_m": exp_m,
                "b_vec": b_vec,
            }
        )
    return in_maps


def kernel(x, W_gate, A, B, W_base, b_base):
    x = np.asarray(x, dtype=np.float32)
    W_gate = np.asarray(W_gate, dtype=np.float32)
    A = np.asarray(A, dtype=np.float32)
    B = np.asarray(B, dtype=np.float32)
    W_base = np.asarray(W_base, dtype=np.float32)
    b_base = np.asarray(b_base, dtype=np.float32)

    if "nc" not in _CACHE:
        _CACHE["nc"] = build_nc()
    nc = _CACHE["nc"]

    in_maps = host_prep(x, W_gate, A, B, W_base, b_base)
    res = run_bass_kernel_spmd(nc, in_maps, core_ids=list(range(CORES)))
    return np.concatenate([res.results[c]["out"] for c in range(CORES)], axis=0)



# revision 2
# speedup vs baseline: 1.0070x; 1.0070x over previous
"""Trainium2 Bass kernel for MoE-with-LoRA-experts (nn_MoE_64098091925598).

Reference computation (N=8192 tokens, D=1024, E=8 experts, R=16, top-2):
    logits  = x @ W_gate.T                      [N, E]
    combine = scatter(softmax(top2(logits)))    [N, E] (2 nonzeros/row)
    moe     = sum_e combine[:,e] * (x @ A_e @ B_e)
    out     = moe + x @ W_base.T + b_base
Data-parallel over tokens across 8 NeuronCores (1024 tokens/core); every
core computes all 8 LoRA experts densely and masks by combine weights.

v2 changes vs baseline (76.4us):
- HAM-aware PE schedule: the PE clock gate re-throttles to 1.2GHz after
  idle; baseline lost ~10us to gaps + cold restarts.  The PE instruction
  stream is now ordered so matmul bursts are back-to-back: front-end
  wave0 -> base j0-3 (both dout halves) -> front-end wave1 -> base j4-7,
  with filler matmuls only at DMA-wait points.
- bf16 for W_base / B / x-copy used in the base+LoRA-combine matmuls and
  for the output (upcast on host).  PE streams at the same 1 cyc/row but
  DMA bytes drop 17.2 -> 8.8 MiB.  The gating logits path stays fp32r
  with identical operands/accumulation order => bitwise-identical top-2
  selection (a flipped near-tie expert would cost ~0.1 rel err).
- Host-packed DRAM layouts so DMA descriptors are 4KB-contiguous runs
  (x, W_base as [128, c, free] with c-pairs per transfer).
- Per-token-chunk [128,1024] output stores (one DMA per 128 tokens).
"""

import numpy as np
import ml_dtypes

import concourse.mybir as mybir
import concourse.tile as tile
from concourse import bacc
from concourse.bass_utils import run_bass_kernel_spmd
from concourse.masks import make_identity

N_TOK, D, E, R, TOPK = 8192, 1024, 8, 16, 2
CORES = 8
NS = N_TOK // CORES  # tokens per core
ER = E * R  # 128, stacked expert-rank dim
DC = D // 128  # 8 contraction chunks
NJ = NS // 128  # 8 token chunks per core
NT = NS // 512  # 2 wide token tiles (waves) per core
JT = NJ // NT  # 4 token chunks per wave
DT = D // 512  # 2 dout tiles
PAIRS = DC // 2  # c-chunk pairs per DMA (4KB descriptor runs)

f32 = mybir.dt.float32
f32r = mybir.dt.float32r
bf16 = mybir.dt.bfloat16

N_WARM = 20  # PE clock-gate warmers while first loads land
FILL_W1 = 2  # fillers before front-end wave1 (x-wave1 DMA margin)

_CACHE: dict = {}


def _kernel_body(nc, tc, dram):
    x0p, x1p, wbp, a_p, b_p, wgp, exp_m, b_vec, out = dram
    xp = (x0p, x1p)

    from contextlib import ExitStack

    ctx = ExitStack()
    pw = ctx.enter_context(tc.tile_pool(name="weights", bufs=1))
    pg = ctx.enter_context(tc.tile_pool(name="gating", bufs=1))
    pmt = ctx.enter_context(tc.tile_pool(name="mmtmp", bufs=2))
    pout = ctx.enter_context(tc.tile_pool(name="outsb", bufs=4))
    ps_tp = ctx.enter_context(tc.tile_pool(name="ps_tp", bufs=2, space="PSUM"))
    ps_mm = ctx.enter_context(tc.tile_pool(name="ps_mm", bufs=2, space="PSUM"))
    ps_out = ctx.enter_context(tc.tile_pool(name="ps_out", bufs=3, space="PSUM"))
    ps_wm = ctx.enter_context(tc.tile_pool(name="ps_wm", bufs=1, space="PSUM"))

    # ---- PE prewarm: garbage matmuls, no data deps, never read -----
    warm_sb = pw.tile([128, 512], bf16, tag="warm")
    warm_ps = ps_wm.tile([128, 512], f32, tag="wm")
    nc.vector.memset(warm_sb, 0.0)

    def fill(n):
        for _ in range(n):
            nc.tensor.matmul(
                warm_ps, warm_sb[:, 0:128], warm_sb, start=True, stop=True
            )

    fill(N_WARM)

    # ---- Load phase: two HWDGE rings, strict priority order ---------
    def ring(i):
        return nc.sync if i % 2 == 0 else nc.scalar

    wg_sb = pw.tile([128, DC, E], f32r, tag="wg")
    nc.scalar.dma_start(out=wg_sb, in_=wgp)
    exp_sb = pw.tile([E, ER], f32r, tag="expand")
    nc.scalar.dma_start(out=exp_sb, in_=exp_m)

    # x wave tiles: [128, 2, 512] fp32 (c-pairs, 4KB rows in DRAM)
    xt = [[None] * PAIRS for _ in range(NT)]
    for t in range(PAIRS):
        tl = pw.tile([128, 2, 512], f32r, tag=f"x0_{t}")
        ring(t).dma_start(out=tl, in_=xp[0][:, 2 * t : 2 * t + 2, :])
        xt[0][t] = tl

    a_sb = pw.tile([128, DC, ER], f32r, tag="a")
    nc.sync.dma_start(out=a_sb, in_=a_p)
    b_sb = pw.tile([ER, D], bf16, tag="bflat")
    nc.scalar.dma_start(out=b_sb, in_=b_p)
    bias_sb = pw.tile([128, D], f32, tag="bias")
    nc.gpsimd.dma_start(out=bias_sb, in_=b_vec.to_broadcast([128, D]))

    # W_base^T tiles: [128, 2, 1024] bf16 (c-pairs, 4KB rows)
    wb = [None] * PAIRS
    for t in range(PAIRS):
        tl = pw.tile([128, 2, D], bf16, tag=f"wb{t}")
        ring(t).dma_start(out=tl, in_=wbp[:, 2 * t : 2 * t + 2, :])
        wb[t] = tl

    for t in range(PAIRS):
        tl = pw.tile([128, 2, 512], f32r, tag=f"x1_{t}")
        ring(t).dma_start(out=tl, in_=xp[1][:, 2 * t : 2 * t + 2, :])
        xt[1][t] = tl

    ident = pw.tile([128, 128], f32, tag="ident")
    make_identity(nc, ident)

    # bf16 copies of x for the base matmuls (gpsimd casts, off hot path)
    xtb = [[None] * PAIRS for _ in range(NT)]
    for w in range(NT):
        for t in range(PAIRS):
            tl = pw.tile([128, 2, 512], bf16, tag=f"xb{w}_{t}")
            nc.gpsimd.tensor_copy(tl, xt[w][t].bitcast(f32))
            xtb[w][t] = tl

    def xchunk(w, c, f32_=True):
        tl = xt[w][c // 2] if f32_ else xtb[w][c // 2]
        return tl[:, c % 2, :]

    # ---- Front-end for one 512-token wave ---------------------------
    HT = [None] * NT  # H^T per wave [ER, 512] bf16

    def front_end(w):
        # logits^T [E, 512]: bitwise-identical operands/order to baseline
        lgT_ps = ps_mm.tile([E, 512], f32, tag="mm")
        for c in range(DC):
            nc.tensor.matmul(
                lgT_ps, wg_sb[:, c, :], xchunk(w, c), start=(c == 0), stop=(c == DC - 1)
            )
        lgT_sb = pg.tile([E, 512], f32, tag=f"lgT{w}")
        nc.vector.tensor_copy(lgT_sb, lgT_ps)

        # token-major logits chunks + sorted top-8 per token (PE transpose)
        lg3 = pg.tile([128, JT, E], f32, tag=f"lg3_{w}")
        mx = pg.tile([128, JT, E], f32, tag=f"mx{w}")
        for r in range(JT):
            tr_ps = ps_tp.tile([128, E], f32, tag="tp")
            nc.tensor.transpose(
                tr_ps, lgT_sb[:, r * 128 : (r + 1) * 128], ident[0:E, 0:E]
            )
            nc.vector.tensor_copy(lg3[:, r, :], tr_ps)
            nc.vector.max(out=mx[:, r, :], in_=lg3[:, r, :])

        # H^T = (A_flat^T @ x^T), PE busy while vector does gating math
        h_ps = ps_mm.tile([ER, 512], f32, tag="mm")
        for c in range(DC):
            nc.tensor.matmul(
                h_ps, a_sb[:, c, :], xchunk(w, c), start=(c == 0), stop=(c == DC - 1)
            )
        h_sb = pmt.tile([ER, 512], f32, tag="hsb")
        nc.vector.tensor_copy(h_sb, h_ps)

        # combine = 1{l==v1}*sigmoid(v1-v2) + 1{l==v2}*sigmoid(v2-v1)
        v1 = mx[:, :, 0:1]
        v2 = mx[:, :, 1:2]
        d21 = pg.tile([128, JT, 1], f32, tag=f"d21_{w}")
        nc.vector.tensor_sub(d21, v2, v1)
        w1 = pg.tile([128, JT, 1], f32, tag=f"w1_{w}")
        w2 = pg.tile([128, JT, 1], f32, tag=f"w2_{w}")
        nc.scalar.activation(w2, d21, mybir.ActivationFunctionType.Sigmoid)
        nc.scalar.activation(w1, d21, mybir.ActivationFunctionType.Sigmoid, scale=-1.0)

        eq1 = pg.tile([128, JT, E], f32, tag=f"eq1_{w}")
        eq2 = pg.tile([128, JT, E], f32, tag=f"eq2_{w}")
        cb = pg.tile([128, JT, E], f32, tag=f"cb{w}")
        bs = [128, JT, E]
        nc.vector.tensor_tensor(eq1, lg3, v1.to_broadcast(bs), mybir.AluOpType.is_equal)
        nc.vector.tensor_tensor(eq2, lg3, v2.to_broadcast(bs), mybir.AluOpType.is_equal)
        nc.vector.tensor_tensor(eq1, eq1, w1.to_broadcast(bs), mybir.AluOpType.mult)
        nc.vector.tensor_tensor(eq2, eq2, w2.to_broadcast(bs), mybir.AluOpType.mult)
        nc.vector.tensor_add(cb, eq1, eq2)

        # combine^T [E, 512] via PE transpose per chunk
        cT_sb = pg.tile([E, 512], f32r, tag=f"cT{w}")
        for r in range(JT):
            cT_ps = ps_tp.tile([E, 128], f32, tag="tp")
            nc.tensor.transpose(cT_ps, cb[:, r, :], ident)
            nc.vector.tensor_copy(cT_sb[:, r * 128 : (r + 1) * 128], cT_ps)

        # expand combine to rank-stacked rows, multiply into H^T (bf16)
        ce_ps = ps_mm.tile([ER, 512], f32, tag="mm")
        nc.tensor.matmul(ce_ps, exp_sb, cT_sb, start=True, stop=True)
        ht = pg.tile([ER, 512], bf16, tag=f"HT{w}")
        nc.vector.tensor_tensor(ht, ce_ps, h_sb, mybir.AluOpType.mult)
        HT[w] = ht

    # ---- Main accumulation for one 128-token chunk ------------------
    # out[jsl, :] = x@Wb^T + H@B_flat + b ; both dout halves then store
    def token_chunk(j):
        w, jr = divmod(j, JT)
        jsl = slice(jr * 128, (jr + 1) * 128)
        out_sb = pout.tile([128, D], bf16, tag="osb")
        for dt in range(DT):
            dsl = slice(dt * 512, (dt + 1) * 512)
            out_ps = ps_out.tile([128, 512], f32, tag="out")
            for c in range(DC):
                nc.tensor.matmul(
                    out_ps,
                    xtb[w][c // 2][:, c % 2, jsl],
                    wb[c // 2][:, c % 2, dsl],
                    start=(c == 0),
                    stop=False,
                )
            nc.tensor.matmul(
                out_ps, HT[w][:, jsl], b_sb[:, dsl], start=False, stop=True
            )
            nc.vector.tensor_add(out_sb[:, dsl], out_ps, bias_sb[:, dsl])
        eng = nc.sync if j % 2 == 0 else nc.scalar
        eng.dma_start(out=out[j * 128 : (j + 1) * 128, :], in_=out_sb)

    front_end(0)
    for j in range(JT):
        token_chunk(j)
    fill(FILL_W1)
    front_end(1)
    for j in range(JT, NJ):
        token_chunk(j)

    ctx.close()


def build_nc():
    nc = bacc.Bacc(
        "TRN2",
        target_bir_lowering=False,
        debug=False,
        enable_asserts=False,
        num_devices=CORES,
    )
    x0p = nc.dram_tensor("x0p", [128, DC, 512], f32, kind="ExternalInput").ap()
    x1p = nc.dram_tensor("x1p", [128, DC, 512], f32, kind="ExternalInput").ap()
    wbp = nc.dram_tensor("wbp", [128, DC, D], bf16, kind="ExternalInput").ap()
    a_p = nc.dram_tensor("a_p", [128, DC, ER], f32, kind="ExternalInput").ap()
    b_p = nc.dram_tensor("b_p", [ER, D], bf16, kind="ExternalInput").ap()
    wgp = nc.dram_tensor("wgp", [128, DC, E], f32, kind="ExternalInput").ap()
    exp_m = nc.dram_tensor("exp_m", [E, ER], f32, kind="ExternalInput").ap()
    b_vec = nc.dram_tensor("b_vec", [1, D], f32, kind="ExternalInput").ap()
    out = nc.dram_tensor("out", [NS, D], bf16, kind="ExternalOutput").ap()

    dram = (
        x0p.bitcast(f32r),
        x1p.bitcast(f32r),
        wbp,
        a_p.bitcast(f32r),
        b_p,
        wgp.bitcast(f32r),
        exp_m.bitcast(f32r),
        b_vec,
        out,
    )
    with tile.TileContext(nc) as tc:
        _kernel_body(nc, tc, dram)
    nc.compile()
    return nc


def host_prep(x, W_gate, A, B, W_base, b_base):
    """Shard + lay out the full inputs into 8 per-core input maps."""
    bft = ml_dtypes.bfloat16
    # x packed [wave][128, c, 512]: xp[w][p, c, n] = x[tok0 + w*512 + n, c*128 + p]
    # W_base^T packed [128, c, dout] bf16
    wbp = np.ascontiguousarray(
        W_base.T.reshape(DC, 128, D).transpose(1, 0, 2)
    ).astype(bft)
    a_p = np.ascontiguousarray(
        A.transpose(1, 0, 2).reshape(DC, 128, ER).transpose(1, 0, 2)
    )
    b_p = np.ascontiguousarray(B.reshape(ER, D)).astype(bft)
    wgp = np.ascontiguousarray(W_gate.T.reshape(DC, 128, E).transpose(1, 0, 2))
    exp_m = np.zeros((E, ER), dtype=np.float32)
    for e in range(E):
        exp_m[e, e * R : (e + 1) * R] = 1.0
    b_vec = np.ascontiguousarray(b_base.reshape(1, D))

    in_maps = []
    for cidx in range(CORES):
        xs = x[cidx * NS : (cidx + 1) * NS]  # [NS, D]
        # [p, c, n] per wave
        xw = xs.T.reshape(DC, 128, NS).transpose(1, 0, 2)
        in_maps.append(
            {
                "x0p": np.ascontiguousarray(xw[:, :, 0:512]),
                "x1p": np.ascontiguousarray(xw[:, :, 512:1024]),
                "wbp": wbp,
                "a_p": a_p,
                "b_p": b_p,
                "wgp": wgp,
                "exp_m": exp_m,
                "b_vec": b_vec,
            }
        )
    return in_maps


def kernel(x, W_gate, A, B, W_base, b_base):
    x = np.asarray(x, dtype=np.float32)
    W_gate = np.asarray(W_gate, dtype=np.float32)
    A = np.asarray(A, dtype=np.float32)
    B = np.asarray(B, dtype=np.float32)
    W_base = np.asarray(W_base, dtype=np.float32)
    b_base = np.asarray(b_base, dtype=np.float32)

    if "nc" not in _CACHE:
        _CACHE["nc"] = build_nc()
    nc = _CACHE["nc"]

    in_maps = host_prep(x, W_gate, A, B, W_base, b_base)
    res = run_bass_kernel_spmd(nc, in_maps, core_ids=list(range(CORES)))
    return np.concatenate(
        [np.asarray(res.results[c]["out"]).astype(np.float32) for c in range(CORES)],
        axis=0,
    )


# revision 4
# speedup vs baseline: 1.0186x; 1.0115x over previous
"""Trainium2 Bass kernel for MoE-with-LoRA-experts (nn_MoE_64098091925598).

Reference computation (N=8192 tokens, D=1024, E=8 experts, R=16, top-2):
    logits  = x @ W_gate.T                      [N, E]
    combine = scatter(softmax(top2(logits)))    [N, E] (2 nonzeros/row)
    moe     = sum_e combine[:,e] * (x @ A_e @ B_e)
    out     = moe + x @ W_base.T + b_base
Data-parallel over tokens across 8 NeuronCores (1024 tokens/core); every
core computes all 8 LoRA experts densely and masks by combine weights.

v3 notes (baseline 76.4us):
- The 16 DMA queues saturate at ~420 GB/s aggregate and each HWDGE ring
  executes its transfers FIFO, so the schedule is built around the byte
  arrival order: x-fp32 wave0 (staggered c-pairs feeding the gating
  matmuls) -> A(bf16) -> x-bf16 wave0 -> W_base dt0 -> dt1 -> x wave1.
- PE HAM clock gate: idle gaps re-throttle the PE to 1.2GHz for ~3.4us.
  Filler matmuls bridge the known DMA-wait points so the PE never goes
  idle between the first gating matmul and the last base matmul.
- bf16 everywhere except the gating logits path, which keeps fp32r with
  baseline-identical operands/accumulation order => bitwise-identical
  top-2 selection (a flipped near-tie would cost ~0.1 rel err).
- x is sent twice from the host (fp32 for logits, bf16 for base/LoRA):
  on-chip casts are far slower (gpsimd ~3.5us per tile) than the DMA.
- Output stored bf16 as [64,1024] halves on both rings, upcast on host.
"""

import numpy as np
import ml_dtypes

import concourse.mybir as mybir
import concourse.tile as tile
from concourse import bacc
from concourse.bass_utils import run_bass_kernel_spmd
from concourse.masks import make_identity

N_TOK, D, E, R, TOPK = 8192, 1024, 8, 16, 2
CORES = 8
NS = N_TOK // CORES  # tokens per core
ER = E * R  # 128, stacked expert-rank dim
DC = D // 128  # 8 contraction chunks
NJ = NS // 128  # 8 token chunks per core
NT = NS // 512  # 2 wide token tiles (waves) per core
JT = NJ // NT  # 4 token chunks per wave
DT = D // 512  # 2 dout tiles

f32 = mybir.dt.float32
f32r = mybir.dt.float32r
bf16 = mybir.dt.bfloat16

N_WARM = 14  # PE clock-gate warmers while first loads land
FILL_XA = 8  # fillers bridging the x0b/a DMA wait after gating trans
FILL_WB = 2  # fillers before the first base matmul (W_base margin)

_CACHE: dict = {}


def _kernel_body(nc, tc, dram):
    x0p, x1p, x0b, x1b, wb0p, wb1p, a_p, b_p, wgp, exp_m, b_vec, out = dram
    xp, xbp, wbp = (x0p, x1p), (x0b, x1b), (wb0p, wb1p)

    from contextlib import ExitStack

    ctx = ExitStack()
    pw = ctx.enter_context(tc.tile_pool(name="weights", bufs=1))
    pg = ctx.enter_context(tc.tile_pool(name="gating", bufs=1))
    pmt = ctx.enter_context(tc.tile_pool(name="mmtmp", bufs=2))
    pout = ctx.enter_context(tc.tile_pool(name="outsb", bufs=4))
    ps_tp = ctx.enter_context(tc.tile_pool(name="ps_tp", bufs=2, space="PSUM"))
    ps_mm = ctx.enter_context(tc.tile_pool(name="ps_mm", bufs=2, space="PSUM"))
    ps_out = ctx.enter_context(tc.tile_pool(name="ps_out", bufs=3, space="PSUM"))
    ps_wm = ctx.enter_context(tc.tile_pool(name="ps_wm", bufs=1, space="PSUM"))

    # ---- PE prewarm: garbage matmuls, no data deps, never read -----
    warm_sb = pw.tile([128, 512], bf16, tag="warm")
    warm_ps = ps_wm.tile([128, 512], f32, tag="wm")
    nc.vector.memset(warm_sb, 0.0)

    def fill(n):
        for _ in range(n):
            nc.tensor.matmul(
                warm_ps, warm_sb[:, 0:128], warm_sb, start=True, stop=True
            )

    fill(N_WARM)

    # ---- Load phase: two HWDGE rings, FIFO per ring, arrival-ordered
    wg_sb = pw.tile([128, DC, E], f32r, tag="wg")
    nc.scalar.dma_start(out=wg_sb, in_=wgp)
    exp_sb = pw.tile([E, ER], f32r, tag="expand")
    nc.scalar.dma_start(out=exp_sb, in_=exp_m)

    # x fp32 wave tiles (logits path), staggered c-pairs on both rings
    xt = [[None] * (DC // 2) for _ in range(NT)]
    for t, eng in ((0, nc.sync), (1, nc.scalar), (2, nc.sync), (3, nc.scalar)):
        tl = pw.tile([128, 2, 512], f32r, tag=f"x0_{t}")
        eng.dma_start(out=tl, in_=xp[0][:, 2 * t : 2 * t + 2, :])
        xt[0][t] = tl

    a_sb = pw.tile([128, DC, ER], bf16, tag="a")
    nc.scalar.dma_start(out=a_sb, in_=a_p)

    # x bf16 wave-0 halves (base/LoRA path)
    xb = [[None] * 2 for _ in range(NT)]
    for h, eng in ((0, nc.sync), (1, nc.scalar)):
        tl = pw.tile([128, 4, 512], bf16, tag=f"xb0_{h}")
        eng.dma_start(out=tl, in_=xbp[0][:, 4 * h : 4 * h + 4, :])
        xb[0][h] = tl

    # W_base^T dt-split halves: [128, c, 512] bf16, 4KB rows
    wb = [[None] * 2 for _ in range(DT)]
    for dt in range(DT):
        for h, eng in ((0, nc.sync), (1, nc.scalar)):
            tl = pw.tile([128, 4, 512], bf16, tag=f"wb{dt}_{h}")
            eng.dma_start(out=tl, in_=wbp[dt][:, 4 * h : 4 * h + 4, :])
            wb[dt][h] = tl
        if dt == 0:
            b_sb = pw.tile([ER, D], bf16, tag="bflat")
            nc.scalar.dma_start(out=b_sb, in_=b_p)

    bias_sb = pw.tile([128, D], f32, tag="bias")
    nc.gpsimd.dma_start(out=bias_sb, in_=b_vec.to_broadcast([128, D]))

    # x wave-1: fp32 halves then bf16 halves
    for h, eng in ((0, nc.sync), (1, nc.scalar)):
        tl = pw.tile([128, 4, 512], f32r, tag=f"x1_{h}")
        eng.dma_start(out=tl, in_=xp[1][:, 4 * h : 4 * h + 4, :])
        xt[1][2 * h] = tl  # holds c 4h..4h+3
    for h, eng in ((0, nc.sync), (1, nc.scalar)):
        tl = pw.tile([128, 4, 512], bf16, tag=f"xb1_{h}")
        eng.dma_start(out=tl, in_=xbp[1][:, 4 * h : 4 * h + 4, :])
        xb[1][h] = tl

    ident = pw.tile([128, 128], f32, tag="ident")
    make_identity(nc, ident)

    def xchunk(w, c):  # fp32 logits operand, chunk c of wave w
        if w == 0:
            return xt[0][c // 2][:, c % 2, :]
        return xt[1][2 * (c // 4)][:, c % 4, :]

    def xbchunk(w, c, jsl=slice(None)):  # bf16 operand
        return xb[w][c // 4][:, c % 4, jsl]

    # ---- Front-end for one 512-token wave ---------------------------
    HT = [None] * NT  # H^T per wave [ER, 512] bf16

    def front_logits(w):
        # logits^T [E, 512]: bitwise-identical operands/order to baseline
        lgT_ps = ps_mm.tile([E, 512], f32, tag="mm")
        for c in range(DC):
            nc.tensor.matmul(
                lgT_ps, wg_sb[:, c, :], xchunk(w, c), start=(c == 0), stop=(c == DC - 1)
            )
        lgT_sb = pg.tile([E, 512], f32, tag=f"lgT{w}")
        nc.vector.tensor_copy(lgT_sb, lgT_ps)

        # token-major logits chunks + sorted top-8 per token (PE transpose)
        lg3 = pg.tile([128, JT, E], f32, tag=f"lg3_{w}")
        mx = pg.tile([128, JT, E], f32, tag=f"mx{w}")
        for r in range(JT):
            tr_ps = ps_tp.tile([128, E], f32, tag="tp")
            nc.tensor.transpose(
                tr_ps, lgT_sb[:, r * 128 : (r + 1) * 128], ident[0:E, 0:E]
            )
            nc.vector.tensor_copy(lg3[:, r, :], tr_ps)
            nc.vector.max(out=mx[:, r, :], in_=lg3[:, r, :])

        # combine = 1{l==v1}*sigmoid(v1-v2) + 1{l==v2}*sigmoid(v2-v1)
        v1 = mx[:, :, 0:1]
        v2 = mx[:, :, 1:2]
        d21 = pg.tile([128, JT, 1], f32, tag=f"d21_{w}")
        nc.vector.tensor_sub(d21, v2, v1)
        w1 = pg.tile([128, JT, 1], f32, tag=f"w1_{w}")
        w2 = pg.tile([128, JT, 1], f32, tag=f"w2_{w}")
        nc.scalar.activation(w2, d21, mybir.ActivationFunctionType.Sigmoid)
        nc.scalar.activation(w1, d21, mybir.ActivationFunctionType.Sigmoid, scale=-1.0)

        eq1 = pg.tile([128, JT, E], f32, tag=f"eq1_{w}")
        eq2 = pg.tile([128, JT, E], f32, tag=f"eq2_{w}")
        cb = pg.tile([128, JT, E], f32, tag=f"cb{w}")
        bs = [128, JT, E]
        nc.vector.tensor_tensor(eq1, lg3, v1.to_broadcast(bs), mybir.AluOpType.is_equal)
        nc.vector.tensor_tensor(eq2, lg3, v2.to_broadcast(bs), mybir.AluOpType.is_equal)
        nc.vector.tensor_tensor(eq1, eq1, w1.to_broadcast(bs), mybir.AluOpType.mult)
        nc.vector.tensor_tensor(eq2, eq2, w2.to_broadcast(bs), mybir.AluOpType.mult)
        nc.vector.tensor_add(cb, eq1, eq2)
        return cb

    def front_moe(w, cb):
        # H^T = (A_flat^T @ x^T) * expand(combine^T)   [bf16 operands]
        h_ps = ps_mm.tile([ER, 512], f32, tag="mm")
        for c in range(DC):
            nc.tensor.matmul(
                h_ps, a_sb[:, c, :], xbchunk(w, c), start=(c == 0), stop=(c == DC - 1)
            )
        h_sb = pmt.tile([ER, 512], f32, tag="hsb")
        nc.vector.tensor_copy(h_sb, h_ps)

        cT_sb = pg.tile([E, 512], f32r, tag=f"cT{w}")
        for r in range(JT):
            cT_ps = ps_tp.tile([E, 128], f32, tag="tp")
            nc.tensor.transpose(cT_ps, cb[:, r, :], ident)
            nc.vector.tensor_copy(cT_sb[:, r * 128 : (r + 1) * 128], cT_ps)

        ce_ps = ps_mm.tile([ER, 512], f32, tag="mm")
        nc.tensor.matmul(ce_ps, exp_sb, cT_sb, start=True, stop=True)
        ht = pg.tile([ER, 512], bf16, tag=f"HT{w}")
        nc.vector.tensor_tensor(ht, ce_ps, h_sb, mybir.AluOpType.mult)
        HT[w] = ht

    # ---- Base+moe accumulation for one (dout half, token chunk) -----
    out_sb = [None] * NJ

    def chunk_pass(dt, j):
        w, jr = divmod(j, JT)
        jsl = slice(jr * 128, (jr + 1) * 128)
        dsl = slice(dt * 512, (dt + 1) * 512)
        if out_sb[j] is None:
            out_sb[j] = pout.tile([128, D], bf16, tag="osb", name=f"osb{j}")
        out_ps = ps_out.tile([128, 512], f32, tag="out")
        for c in range(DC):
            nc.tensor.matmul(
                out_ps,
                xbchunk(w, c, jsl),
                wb[dt][c // 4][:, c % 4, :],
                start=(c == 0),
                stop=False,
            )
        nc.tensor.matmul(out_ps, HT[w][:, jsl], b_sb[:, dsl], start=False, stop=True)
        nc.vector.tensor_add(out_sb[j][:, dsl], out_ps, bias_sb[:, dsl])
        if dt == DT - 1:
            j0 = j * 128
            nc.sync.dma_start(out=out[j0 : j0 + 64, :], in_=out_sb[j][0:64, :])
            nc.scalar.dma_start(out=out[j0 + 64 : j0 + 128, :], in_=out_sb[j][64:128, :])

    # ---- Program order (PE continuity is the whole game) ------------
    cb0 = front_logits(0)
    fill(FILL_XA)  # bridge the a/x0b DMA wait; vector does gating now
    front_moe(0, cb0)
    fill(FILL_WB)
    for j in range(JT):
        chunk_pass(0, j)
    cb1 = front_logits(1)  # x wave-1 fp32 lands about now
    front_moe(1, cb1)
    for j in range(JT):
        chunk_pass(1, j)
    for j in range(JT, NJ):
        chunk_pass(0, j)
        chunk_pass(1, j)

    ctx.close()


def build_nc():
    nc = bacc.Bacc(
        "TRN2",
        target_bir_lowering=False,
        debug=False,
        enable_asserts=False,
        num_devices=CORES,
    )
    x0p = nc.dram_tensor("x0p", [128, DC, 512], f32, kind="ExternalInput").ap()
    x1p = nc.dram_tensor("x1p", [128, DC, 512], f32, kind="ExternalInput").ap()
    x0b = nc.dram_tensor("x0b", [128, DC, 512], bf16, kind="ExternalInput").ap()
    x1b = nc.dram_tensor("x1b", [128, DC, 512], bf16, kind="ExternalInput").ap()
    wb0p = nc.dram_tensor("wb0p", [128, DC, 512], bf16, kind="ExternalInput").ap()
    wb1p = nc.dram_tensor("wb1p", [128, DC, 512], bf16, kind="ExternalInput").ap()
    a_p = nc.dram_tensor("a_p", [128, DC, ER], bf16, kind="ExternalInput").ap()
    b_p = nc.dram_tensor("b_p", [ER, D], bf16, kind="ExternalInput").ap()
    wgp = nc.dram_tensor("wgp", [128, DC, E], f32, kind="ExternalInput").ap()
    exp_m = nc.dram_tensor("exp_m", [E, ER], f32, kind="ExternalInput").ap()
    b_vec = nc.dram_tensor("b_vec", [1, D], f32, kind="ExternalInput").ap()
    out = nc.dram_tensor("out", [NS, D], bf16, kind="ExternalOutput").ap()

    dram = (
        x0p.bitcast(f32r),
        x1p.bitcast(f32r),
        x0b,
        x1b,
        wb0p,
        wb1p,
        a_p,
        b_p,
        wgp.bitcast(f32r),
        exp_m.bitcast(f32r),
        b_vec,
        out,
    )
    with tile.TileContext(nc) as tc:
        _kernel_body(nc, tc, dram)
    nc.compile()
    return nc


def host_prep(x, W_gate, A, B, W_base, b_base):
    """Shard + lay out the full inputs into 8 per-core input maps."""
    bft = ml_dtypes.bfloat16
    wbT = W_base.T.reshape(DC, 128, D).transpose(1, 0, 2).astype(bft)  # [p, c, d]
    wb0p = np.ascontiguousarray(wbT[:, :, 0:512])
    wb1p = np.ascontiguousarray(wbT[:, :, 512:1024])
    a_p = np.ascontiguousarray(
        A.transpose(1, 0, 2).reshape(DC, 128, ER).transpose(1, 0, 2).astype(bft)
    )
    b_p = np.ascontiguousarray(B.reshape(ER, D)).astype(bft)
    wgp = np.ascontiguousarray(W_gate.T.reshape(DC, 128, E).transpose(1, 0, 2))
    exp_m = np.zeros((E, ER), dtype=np.float32)
    for e in range(E):
        exp_m[e, e * R : (e + 1) * R] = 1.0
    b_vec = np.ascontiguousarray(b_base.reshape(1, D))

    in_maps = []
    for cidx in range(CORES):
        xs = x[cidx * NS : (cidx + 1) * NS]  # [NS, D]
        xw = xs.T.reshape(DC, 128, NS).transpose(1, 0, 2)  # [p, c, n]
        x0 = np.ascontiguousarray(xw[:, :, 0:512])
        x1 = np.ascontiguousarray(xw[:, :, 512:1024])
        in_maps.append(
            {
                "x0p": x0,
                "x1p": x1,
                "x0b": x0.astype(bft),
                "x1b": x1.astype(bft),
                "wb0p": wb0p,
                "wb1p": wb1p,
                "a_p": a_p,
                "b_p": b_p,
                "wgp": wgp,
                "exp_m": exp_m,
                "b_vec": b_vec,
            }
        )
    return in_maps


def kernel(x, W_gate, A, B, W_base, b_base):
    x = np.asarray(x, dtype=np.float32)
    W_gate = np.asarray(W_gate, dtype=np.float32)
    A = np.asarray(A, dtype=np.float32)
    B = np.asarray(B, dtype=np.float32)
    W_base = np.asarray(W_base, dtype=np.float32)
    b_base = np.asarray(b_base, dtype=np.float32)

    if "nc" not in _CACHE:
        _CACHE["nc"] = build_nc()
    nc = _CACHE["nc"]

    in_maps = host_prep(x, W_gate, A, B, W_base, b_base)
    res = run_bass_kernel_spmd(nc, in_maps, core_ids=list(range(CORES)))
    return np.concatenate(
        [np.asarray(res.results[c]["out"]).astype(np.float32) for c in range(CORES)],
        axis=0,
    )
